# revision 1
# baseline (speedup 1.0000x reference)
"""ClusterGNN Trainium2 kernel.

Data-parallel over bags: 16 bags -> 8 cores x 2 bags. Each core runs the
full per-bag pipeline:

  h  = relu(x @ We + be)                       (encoder)
  s1 = segment_sum(u1[src], dst), u1 = h @ Wl1 (gather + scatter-add)
  g1 = relu(s1 / deg + h @ Wr1 + bl1)
  s2 = segment_sum(u2[src], dst), u2 = g1 @ Wl2
  g2 = relu(s2 / deg + g1 @ Wr2 + bl2)
  emb = sum_n g2[n]     (the diff-pool softmax over a size-1 axis is == 1)
  out = relu(emb @ Wc1 + bc1) @ Wc2 + bc2

Aggregation uses SWDGE dma_gather (DRAM feature table -> edge-major SBUF
chunks) and dma_scatter_add (SBUF chunks -> parity-split SBUF accumulators
keyed by dst). Dense algebra runs on the PE in bf16 with f32 PSUM.
"""

from contextlib import ExitStack

import numpy as np

import concourse.bass as bass
import concourse.tile as tile
from concourse import bacc, mybir
from concourse.bass_utils import run_bass_kernel_spmd
from concourse.masks import make_identity

# Problem shape (hardcoded per contract).
B, N, E, D_IN, D_ENC, D_FC, N_CLS = 16, 5000, 160000, 128, 256, 128, 2
M_CORES = 8
P = 128

FD = mybir.dt.float32
BF = mybir.dt.bfloat16
I16 = mybir.dt.int16

ts = bass.ts

# Each gather/scatter call must stay under the SWDGE ucode ring capacity of
# ~1024 descriptors — 2000-token calls crash the device even with a larger
# dynamic_dma_scratch_size (the ucode cap is fixed). CLS=160 -> 1000-token
# calls, and max_degree (57) <= CLS keeps per-call indices unique.
FULL_CFG = dict(BPC=B // M_CORES, N=N, E=E, CLS=160)
DMA_SCRATCH = 16384


def _ceil_div(a, b):
    return (a + b - 1) // b


def _class_geometry(E, CLS):
    """Wave-class geometry: host sorts edges by dst and deals them into CLS
    strided classes; within a class any two edges are >= CLS apart in sorted
    order, so (max_deg <= CLS) implies all dst in a class are distinct ->
    dma_scatter_add sees unique indices (its HW RMW races on duplicates)."""
    assert E % CLS == 0
    VC = E // CLS  # valid tokens per class
    CSZ = _ceil_div(VC, 16) * 16  # idx slots per class (pad with -1)
    BTOK = _ceil_div(CSZ, P) * P  # buffer token slots per class
    return VC, CSZ, BTOK


def build_kernel(cfg=None):
    cfg = cfg or FULL_CFG
    BPC, N, E, CLS = cfg["BPC"], cfg["N"], cfg["E"], cfg["CLS"]
    NT = (N + P - 1) // P
    NP = NT * P
    VC, CSZ, BTOK = _class_geometry(E, CLS)
    assert CSZ <= DMA_SCRATCH // 16, "call exceeds SWDGE descriptor ring"
    nc = bacc.Bacc("TRN2", dynamic_dma_scratch_size=DMA_SCRATCH)

    # ---- I/O ----
    xT = nc.dram_tensor("xT", [BPC, D_IN, N], FD, kind="ExternalInput")
    srcw = nc.dram_tensor("srcw", [BPC, P, CLS * CSZ // 16], I16, kind="ExternalInput")
    dstw = nc.dram_tensor("dstw", [BPC, P, CLS * CSZ // 16], I16, kind="ExternalInput")
    We_d = nc.dram_tensor("We", [D_IN, D_ENC], FD, kind="ExternalInput")
    be_d = nc.dram_tensor("be", [1, D_ENC], FD, kind="ExternalInput")
    Wl1_d = nc.dram_tensor("Wl1", [D_ENC, D_ENC], FD, kind="ExternalInput")
    bl1_d = nc.dram_tensor("bl1", [1, D_ENC], FD, kind="ExternalInput")
    Wr1_d = nc.dram_tensor("Wr1", [D_ENC, D_ENC], FD, kind="ExternalInput")
    Wl2_d = nc.dram_tensor("Wl2", [D_ENC, D_ENC], FD, kind="ExternalInput")
    bl2_d = nc.dram_tensor("bl2", [1, D_ENC], FD, kind="ExternalInput")
    Wr2_d = nc.dram_tensor("Wr2", [D_ENC, D_ENC], FD, kind="ExternalInput")
    Wc1_d = nc.dram_tensor("Wc1", [D_ENC, D_FC], FD, kind="ExternalInput")
    bc1_d = nc.dram_tensor("bc1", [1, D_FC], FD, kind="ExternalInput")
    Wc2_d = nc.dram_tensor("Wc2", [D_FC, N_CLS], FD, kind="ExternalInput")
    bc2_d = nc.dram_tensor("bc2", [1, N_CLS], FD, kind="ExternalInput")
    out_d = nc.dram_tensor("out", [BPC, N_CLS], FD, kind="ExternalOutput")

    # Per (bag, layer) feature tables for the edge gather.
    tabs = [
        [nc.dram_tensor(f"tab_b{b}_l{l}", [NP, D_ENC], BF) for l in range(2)]
        for b in range(BPC)
    ]

    with tile.TileContext(nc) as tc, ExitStack() as ctx:
        wpool = ctx.enter_context(tc.tile_pool(name="weights", bufs=1))
        big = ctx.enter_context(tc.tile_pool(name="big", bufs=1))
        accp = ctx.enter_context(tc.tile_pool(name="acc", bufs=1))
        chp = ctx.enter_context(tc.tile_pool(name="chunks", bufs=1))
        xp = ctx.enter_context(tc.tile_pool(name="xpieces", bufs=2))
        stp = ctx.enter_context(tc.tile_pool(name="stage", bufs=2))
        htp = ctx.enter_context(tc.tile_pool(name="ht", bufs=2))
        idxp = ctx.enter_context(tc.tile_pool(name="idx", bufs=4))
        smp = ctx.enter_context(tc.tile_pool(name="small", bufs=2))
        psE = ctx.enter_context(tc.tile_pool(name="psE", bufs=2, space="PSUM"))
        psT = ctx.enter_context(tc.tile_pool(name="psT", bufs=2, space="PSUM"))
        psU = ctx.enter_context(tc.tile_pool(name="psU", bufs=2, space="PSUM"))
        psR = ctx.enter_context(tc.tile_pool(name="psR", bufs=2, space="PSUM"))

        # ---- constants & weights ----
        ident_b = wpool.tile([P, P], BF, tag="ident_b")
        make_identity(nc, ident_b[:])
        ident_f = wpool.tile([P, P], FD, tag="ident_f")
        make_identity(nc, ident_f[:])

        ones_row_f = wpool.tile([1, P], FD, tag="ones_row_f")
        nc.vector.memset(ones_row_f[:], 1.0)
        ones_row_b = wpool.tile([1, P], BF, tag="ones_row_b")
        nc.vector.memset(ones_row_b[:], 1.0)
        ones_col_b = wpool.tile([P, 1], BF, tag="ones_col_b")
        nc.vector.memset(ones_col_b[:], 1.0)

        We_t = wpool.tile([P, D_ENC], FD, tag="We")
        nc.sync.dma_start(We_t[:], We_d[:, :])
        be_t = wpool.tile([1, D_ENC], FD, tag="be")
        nc.sync.dma_start(be_t[:], be_d[:, :])

        def load_w_bf(dram, cols, tag):
            tiles = []
            for c in range(2):
                t = wpool.tile([P, cols], BF, tag=f"{tag}{c}")
                # SWDGE cast f32 -> bf16 during DMA.
                nc.gpsimd.dma_start(out=t[:], in_=dram[c * P:(c + 1) * P, :])
                tiles.append(t)
            return tiles

        Wl1_t = load_w_bf(Wl1_d, D_ENC, "Wl1")
        Wr1_t = load_w_bf(Wr1_d, D_ENC, "Wr1")
        Wl2_t = load_w_bf(Wl2_d, D_ENC, "Wl2")
        Wr2_t = load_w_bf(Wr2_d, D_ENC, "Wr2")

        bl1_t = wpool.tile([1, D_ENC], BF, tag="bl1")
        nc.gpsimd.dma_start(out=bl1_t[:], in_=bl1_d[:, :])
        bl2_t = wpool.tile([1, D_ENC], BF, tag="bl2")
        nc.gpsimd.dma_start(out=bl2_t[:], in_=bl2_d[:, :])

        Wc1_t = []
        for c in range(2):
            t = wpool.tile([P, D_FC], FD, tag=f"Wc1{c}")
            nc.sync.dma_start(t[:], Wc1_d[c * P:(c + 1) * P, :])
            Wc1_t.append(t)
        Wc2_t = wpool.tile([P, N_CLS], FD, tag="Wc2")
        nc.sync.dma_start(Wc2_t[:], Wc2_d[:, :])
        bc1_t = wpool.tile([1, D_FC], FD, tag="bc1")
        nc.sync.dma_start(bc1_t[:], bc1_d[:, :])
        bc2_t = wpool.tile([1, N_CLS], FD, tag="bc2")
        nc.sync.dma_start(bc2_t[:], bc2_d[:, :])

        # ones chunk for degree scatter (elem 128 bf16 = 256B)
        ones_ch = wpool.tile([P, (BTOK // P) * P], BF, tag="ones_ch")

        # gather/scatter token buffers (explicit ring, zeroed once so the
        # pad-token slots stay finite)
        NBUF = 4
        bufs = [
            chp.tile([P, (BTOK // P) * D_ENC], BF, tag=f"chunk{i}", name=f"chunk{i}")
            for i in range(NBUF)
        ]
        for bft in bufs:
            nc.vector.memset(bft[:], 0.0)

        for bag in range(BPC):
            # ---------------- encoder: h = relu(x @ We + be) ----------------
            h = big.tile([P, NT * D_ENC], BF, tag="h")
            TPB = 10  # node tiles per x piece
            for t in range(NT):
                if t % TPB == 0:
                    xpiece = xp.tile([P, TPB * P], FD, tag="xpiece")
                    lo = t * P
                    hi = min(N, (t + TPB) * P)
                    if hi - lo < TPB * P:
                        nc.vector.memset(xpiece[:], 0.0)
                    nc.sync.dma_start(
                        xpiece[:, : hi - lo], xT[bag, :, lo:hi]
                    )
                ps = psE.tile([P, D_ENC], FD, tag="psE")
                nc.tensor.matmul(
                    ps[:], lhsT=xpiece[:, ts(t % TPB, P)], rhs=We_t[:],
                    start=True, stop=False,
                )
                nc.tensor.matmul(
                    ps[:], lhsT=ones_row_f[:1, :], rhs=be_t[:1, :],
                    start=False, stop=True,
                )
                nc.scalar.activation(
                    h[:, ts(t, D_ENC)], ps[:],
                    mybir.ActivationFunctionType.Relu,
                )

            # ---------------- degree (shared by both layers) ----------------
            nc.vector.memset(ones_ch[:], 1.0)
            deg = [
                [
                    accp.tile(
                        [P, (NT // 2) * P], BF,
                        tag=f"deg{i}{j}", name=f"deg{i}{j}",
                    )
                    for j in range(2)
                ]
                for i in range(2)
            ]
            for i in range(2):
                for j in range(2):
                    nc.vector.memset(deg[i][j][:], 0.0)
            ones_v = ones_ch[:].rearrange("p (c f) -> p c f", f=P)
            for c in range(CLS):
                c0 = c * (CSZ // 16)
                idx_d = idxp.tile([P, CSZ // 16], I16, tag="idx_d")
                nc.scalar.dma_start(
                    idx_d[:], dstw[bag, :, c0:c0 + CSZ // 16]
                )
                pair = deg[c % 2]
                nc.gpsimd.dma_scatter_add(
                    pair[0][:],
                    ones_v[:, :, :],
                    idx_d[:],
                    CSZ,
                    VC,
                    P,
                    sbuf_tokens_per_rank=P,
                    parity_reg=0,
                    out_ap_other=pair[1][:],
                )

            # rec[:, t] = 1 / max(deg_tile_t, 1)
            rec = smp.tile([P, NT], FD, tag="rec")
            for t in range(NT):
                col = (t // 2) * P
                dA = deg[0][t % 2][:, col:col + 1]
                dB = deg[1][t % 2][:, col:col + 1]
                tmp1 = smp.tile([P, 1], FD, tag="degtmp")
                nc.vector.tensor_add(tmp1[:], dA, dB)
                nc.vector.tensor_scalar_max(tmp1[:], tmp1[:], 1.0)
                nc.vector.reciprocal(rec[:, t:t + 1], tmp1[:])

            gprev = h  # features feeding the current layer
            for layer in range(2):
                Wl_t = (Wl1_t, Wl2_t)[layer]
                Wr_t = (Wr1_t, Wr2_t)[layer]
                bl_t = (bl1_t, bl2_t)[layer]
                tab = tabs[bag][layer]
                tab_v = tab[:].rearrange("(t p) f -> p t f", p=P)

                # ---- u = gprev @ Wl -> table; r = gprev @ Wr + bl ----
                r = big.tile([P, NT * D_ENC], BF, tag="r")
                GRP = 8
                for t in range(NT):
                    pst = psT.tile([P, D_ENC], BF, tag="psTb")
                    nc.tensor.transpose(
                        pst[:, 0:P], gprev[:, t * D_ENC:t * D_ENC + P], ident_b[:]
                    )
                    nc.tensor.transpose(
                        pst[:, P:D_ENC],
                        gprev[:, t * D_ENC + P:(t + 1) * D_ENC],
                        ident_b[:],
                    )
                    hT = htp.tile([P, D_ENC], BF, tag="hT")
                    nc.vector.tensor_copy(hT[:], pst[:])

                    psu = psU.tile([P, D_ENC], FD, tag="psU")
                    nc.tensor.matmul(
                        psu[:], lhsT=hT[:, 0:P], rhs=Wl_t[0][:],
                        start=True, stop=False,
                    )
                    nc.tensor.matmul(
                        psu[:], lhsT=hT[:, P:D_ENC], rhs=Wl_t[1][:],
                        start=False, stop=True,
                    )
                    psr = psR.tile([P, D_ENC], FD, tag="psR")
                    nc.tensor.matmul(
                        psr[:], lhsT=hT[:, 0:P], rhs=Wr_t[0][:],
                        start=True, stop=False,
                    )
                    nc.tensor.matmul(
                        psr[:], lhsT=hT[:, P:D_ENC], rhs=Wr_t[1][:],
                        start=False, stop=False,
                    )
                    nc.tensor.matmul(
                        psr[:], lhsT=ones_row_b[:1, :], rhs=bl_t[:1, :],
                        start=False, stop=True,
                    )
                    if t % GRP == 0:
                        stage = stp.tile([P, GRP * D_ENC], BF, tag="stage")
                    nc.scalar.copy(stage[:, ts(t % GRP, D_ENC)], psu[:])
                    nc.vector.tensor_copy(r[:, ts(t, D_ENC)], psr[:])
                    if t % GRP == GRP - 1 or t == NT - 1:
                        g0 = t - (t % GRP)
                        nc.sync.dma_start(
                            tab_v[:, g0:t + 1, :],
                            stage[:, : (t - g0 + 1) * D_ENC].rearrange(
                                "p (c f) -> p c f", f=D_ENC
                            ),
                        )

                # ---- gather u[src] / scatter-add by dst ----
                acc = [
                    [
                        accp.tile(
                            [P, (NT // 2) * D_ENC], BF,
                            tag=f"acc{i}{j}", name=f"acc{i}{j}",
                        )
                        for j in range(2)
                    ]
                    for i in range(2)
                ]
                for i in range(2):
                    for j in range(2):
                        nc.vector.memset(acc[i][j][:], 0.0)
                for c in range(CLS):
                    c0 = c * (CSZ // 16)
                    idx_s = idxp.tile([P, CSZ // 16], I16, tag="idx_s")
                    nc.scalar.dma_start(
                        idx_s[:], srcw[bag, :, c0:c0 + CSZ // 16]
                    )
                    idx_d = idxp.tile([P, CSZ // 16], I16, tag="idx_d2")
                    nc.scalar.dma_start(
                        idx_d[:], dstw[bag, :, c0:c0 + CSZ // 16]
                    )
                    buf = bufs[c % NBUF]
                    buf_v = buf[:].rearrange("p (c f) -> p c f", f=D_ENC)
                    nc.gpsimd.dma_gather(
                        buf_v[:, :, :],
                        tab[:, :],
                        idx_s[:],
                        CSZ,
                        VC,
                        D_ENC,
                    )
                    pair = acc[c % 2]
                    nc.gpsimd.dma_scatter_add(
                        pair[0][:],
                        buf_v[:, :, :],
                        idx_d[:],
                        CSZ,
                        VC,
                        D_ENC,
                        sbuf_tokens_per_rank=P,
                        parity_reg=0,
                        out_ap_other=pair[1][:],
                    )

                # ---- g = relu(s * rec + r) ----
                gnew = big.tile([P, NT * D_ENC], BF, tag="g")
                for t in range(NT):
                    j = t % 2  # parity: even slot -> own (j=0)
                    col = (t // 2) * D_ENC
                    sA = acc[0][j][:, col:col + D_ENC]
                    sB = acc[1][j][:, col:col + D_ENC]
                    sm = smp.tile([P, D_ENC], FD, tag="sm")
                    nc.vector.tensor_add(sm[:], sA, sB)
                    nc.vector.tensor_scalar_mul(sm[:], sm[:], rec[:, t:t + 1])
                    nc.vector.tensor_add(sm[:], sm[:], r[:, ts(t, D_ENC)])
                    nc.scalar.activation(
                        gnew[:, ts(t, D_ENC)], sm[:],
                        mybir.ActivationFunctionType.Relu,
                    )
                gprev = gnew

            # ---------------- pooling + classifier ----------------
            g2 = gprev
            pse = psE.tile([1, D_ENC], FD, tag="psE")
            for t in range(NT):
                rows = min(P, N - t * P)
                nc.tensor.matmul(
                    pse[:1, :],
                    lhsT=ones_col_b[:rows, :],
                    rhs=g2[:rows, ts(t, D_ENC)],
                    start=(t == 0),
                    stop=(t == NT - 1),
                )
            emb_s = smp.tile([1, D_ENC], FD, tag="emb")
            nc.vector.tensor_copy(emb_s[:], pse[:1, :])

            pst2 = psE.tile([P, 2], FD, tag="psE")
            nc.tensor.transpose(pst2[:, 0:1], emb_s[:1, 0:P], ident_f[:1, :1])
            nc.tensor.transpose(pst2[:, 1:2], emb_s[:1, P:D_ENC], ident_f[:1, :1])
            embT = smp.tile([P, 2], FD, tag="embT")
            nc.vector.tensor_copy(embT[:], pst2[:])

            ps1 = psU.tile([1, D_FC], FD, tag="psU")
            nc.tensor.matmul(ps1[:1, :], lhsT=embT[:, 0:1], rhs=Wc1_t[0][:],
                             start=True, stop=False)
            nc.tensor.matmul(ps1[:1, :], lhsT=embT[:, 1:2], rhs=Wc1_t[1][:],
                             start=False, stop=False)
            nc.tensor.matmul(ps1[:1, :], lhsT=ones_row_f[:1, :1], rhs=bc1_t[:1, :],
                             start=False, stop=True)
            h1_s = smp.tile([1, D_FC], FD, tag="h1")
            nc.scalar.activation(h1_s[:], ps1[:1, :],
                                 mybir.ActivationFunctionType.Relu)

            pst3 = psE.tile([P, 1], FD, tag="psE")
            nc.tensor.transpose(pst3[:, 0:1], h1_s[:1, :], ident_f[:1, :1])
            h1T = smp.tile([P, 1], FD, tag="h1T")
            nc.vector.tensor_copy(h1T[:], pst3[:])

            pso = psR.tile([1, N_CLS], FD, tag="psR")
            nc.tensor.matmul(pso[:1, :], lhsT=h1T[:, 0:1], rhs=Wc2_t[:],
                             start=True, stop=False)
            nc.tensor.matmul(pso[:1, :], lhsT=ones_row_f[:1, :1], rhs=bc2_t[:1, :],
                             start=False, stop=True)
            out_s = smp.tile([1, N_CLS], FD, tag="outs")
            nc.vector.tensor_copy(out_s[:], pso[:1, :])
            nc.sync.dma_start(out_d[bag:bag + 1, :], out_s[:1, :])

    nc.finalize()
    return nc


_NC_CACHE = {}


def _get_nc():
    if "nc" not in _NC_CACHE:
        _NC_CACHE["nc"] = build_kernel()
    return _NC_CACHE["nc"]


def prep_edges(src, dst, E, CLS):
    """Per-bag edge layout: sort by dst, deal into CLS strided wave classes
    (class c takes sorted positions c, c+CLS, ...), pad each class with -1 to
    a multiple of 16, then 16-wrap + replicate x8 for the SWDGE idx format.

    src/dst: [NB, E] int arrays. Returns (srcw, dstw): [NB, 128, CLS*CSZ//16]
    int16 arrays. Requires per-bag max degree <= CLS (makes every scatter
    call's dst indices unique).
    """
    NB = src.shape[0]
    VC, CSZ, _ = _class_geometry(E, CLS)
    srcw = np.full((NB, CLS, CSZ), -1, np.int16)
    dstw = np.full((NB, CLS, CSZ), -1, np.int16)
    for b in range(NB):
        d = dst[b]
        assert np.bincount(d).max() <= CLS, "max degree exceeds wave count"
        order = np.argsort(d, kind="stable")
        s_s, d_s = src[b][order], d[order]
        # class c gets sorted positions c::CLS
        srcw[b, :, :VC] = s_s.reshape(VC, CLS).T
        dstw[b, :, :VC] = d_s.reshape(VC, CLS).T
    # 16-wrap per class: token i of class c -> [i % 16, c*CSZ//16 + i//16]
    def wrap(a):
        w = a.reshape(NB, CLS, CSZ // 16, 16).transpose(0, 3, 1, 2)
        w = w.reshape(NB, 16, CLS * (CSZ // 16))
        return np.ascontiguousarray(np.tile(w, (1, 8, 1)))

    return wrap(srcw), wrap(dstw)


def kernel(**inputs):
    x = np.asarray(inputs["x"], np.float32)  # [B, N, D_IN]
    ei = np.asarray(inputs["edge_index"])  # [B, 2, E] int64

    xT = np.ascontiguousarray(x.transpose(0, 2, 1))  # [B, D_IN, N]
    src = np.ascontiguousarray(ei[:, 0, :]).astype(np.int16)
    dst = np.ascontiguousarray(ei[:, 1, :]).astype(np.int16)
    srcw, dstw = prep_edges(src, dst, E, FULL_CFG["CLS"])

    def w32(name):
        return np.ascontiguousarray(np.asarray(inputs[name], np.float32))

    def row(name):
        return np.ascontiguousarray(
            np.asarray(inputs[name], np.float32).reshape(1, -1)
        )

    nc = _get_nc()
    BPC = B // M_CORES
    in_maps = []
    trace = bool(globals().get("TRACE", False))
    for core in range(M_CORES):
        sl = slice(core * BPC, (core + 1) * BPC)
        in_maps.append(
            {
                "xT": np.ascontiguousarray(xT[sl]),
                "srcw": np.ascontiguousarray(srcw[sl]),
                "dstw": np.ascontiguousarray(dstw[sl]),
                "We": w32("We"),
                "be": row("be"),
                "Wl1": w32("Wl1"),
                "bl1": row("bl1"),
                "Wr1": w32("Wr1"),
                "Wl2": w32("Wl2"),
                "bl2": row("bl2"),
                "Wr2": w32("Wr2"),
                "Wc1": w32("Wc1"),
                "bc1": row("bc1"),
                "Wc2": w32("Wc2"),
                "bc2": row("bc2"),
            }
        )
    import time as _time

    _t0 = _time.perf_counter()
    res = run_bass_kernel_spmd(
        nc, in_maps, core_ids=list(range(M_CORES)), trace=trace
    )
    globals()["LAST_RUN_WALL_NS"] = int((_time.perf_counter() - _t0) * 1e9)
    if trace:
        globals()["LAST_EXEC_TIME_NS"] = res.exec_time_ns
    out = np.concatenate([r["out"] for r in res.results], axis=0)
    return out.astype(np.float32)

